# revision 4
# baseline (speedup 1.0000x reference)
"""Trainium2 Bass kernel for nn_MixtureOfExperts_58265526337941.

MoE: gating MLP (D->GD->GD->E, softmax) + E experts (D->H->H/2->2),
combined with gate weights.  B=16384, D=512, E=16, H=1024, GD=256.

Strategy: data-parallel over the batch across 8 cores (B_loc=2048 each),
all params replicated, no collectives.  On-chip layout is
"feature-transposed": activations live as [feature-partition, batch-free]
tiles so the contraction dim is always on partitions and no transposes
are needed between layers.  x is transposed on the host (part of input
sharding) so the kernel DMAs xT directly.

Expert layer l3 produces eoT [2, batch] per expert; these are gathered
(via SBUF->SBUF DMA partition shift) into eo_all [32, batch] with expert
e at partitions 2e:2e+2.  The gating softmax is computed un-normalized
(exp only, duplicated per output col -> exp2T [32, batch]); the combine
is then:  out[o,b] = (sum_p S32[p,o] * exp2T[p,b]*eo_all[p,b]) * recip[b]
with S32 the 0/1 interleave-selection matrix and recip = 1/sum_e exp.
b3 is folded into the eo evacuation (activation bias), so it is gate-
weighted exactly like the reference.

Matmuls run as float32r (full PE rate at N=512); everything else fp32.
"""

import os
import sys
import numpy as np
from contextlib import ExitStack

for _p in ("/opt/trn_rl_repo",):
    if _p not in sys.path and os.path.isdir(_p):
        sys.path.insert(0, _p)

# Problem constants (hardcoded per contract)
B, D, E, H, GD = 16384, 512, 16, 1024, 256
H2 = H // 2
OUT = 2
N_CORES = 8
B_LOC = B // N_CORES  # 2048
P = 128
CH = 512  # batch chunk = matmul moving free dim = one fp32 PSUM bank


def build_moe(b_loc=B_LOC):
    import concourse.bacc as bacc
    import concourse.mybir as mybir
    from concourse import tile

    f32 = mybir.dt.float32
    f32r = mybir.dt.float32r
    AF = mybir.ActivationFunctionType
    ALU = mybir.AluOpType

    NCH = b_loc // CH
    DK = D // P    # 4  k-tiles over D
    HM = H // P    # 8  m-tiles over H
    H2M = H2 // P  # 4  m-tiles over H2
    GDK = GD // P  # 2

    nc = bacc.Bacc("TRN2", target_bir_lowering=False, debug=False)

    # ---- DRAM I/O -------------------------------------------------------
    xT_d = nc.dram_tensor("xT", [D, b_loc], f32r, kind="ExternalInput")
    W1_d = nc.dram_tensor("W1", [E, D, H], f32r, kind="ExternalInput")
    b1_d = nc.dram_tensor("b1", [E, H], f32, kind="ExternalInput")
    W2_d = nc.dram_tensor("W2", [E, H, H2], f32r, kind="ExternalInput")
    b2_d = nc.dram_tensor("b2", [E, H2], f32, kind="ExternalInput")
    W3_d = nc.dram_tensor("W3", [E, H2, OUT], f32r, kind="ExternalInput")
    b3T_d = nc.dram_tensor("b3T", [OUT, E], f32, kind="ExternalInput")
    G1_d = nc.dram_tensor("G1", [D, GD], f32r, kind="ExternalInput")
    g1_d = nc.dram_tensor("g1", [GD], f32, kind="ExternalInput")
    G2_d = nc.dram_tensor("G2", [GD, GD], f32r, kind="ExternalInput")
    g2_d = nc.dram_tensor("g2", [GD], f32, kind="ExternalInput")
    G3d_d = nc.dram_tensor("G3d", [GD, 2 * E], f32r, kind="ExternalInput")
    g3d_d = nc.dram_tensor("g3d", [2 * E], f32, kind="ExternalInput")
    S32_d = nc.dram_tensor("S32", [2 * E, OUT], f32r, kind="ExternalInput")
    H32_d = nc.dram_tensor("H32", [2 * E, OUT], f32r, kind="ExternalInput")
    out_d = nc.dram_tensor("out", [b_loc, OUT], f32, kind="ExternalOutput")

    def mm(ps, lhsT, rhs, start, stop):
        nc.tensor.matmul(ps, lhsT.bitcast(f32r), rhs.bitcast(f32r),
                         start=start, stop=stop)

    with tile.TileContext(nc) as tc, ExitStack() as ctx:
        const = ctx.enter_context(tc.tile_pool(name="const", bufs=1))

        # Persistent tiles
        xT = const.tile([P, DK, b_loc], f32r)
        nc.sync.dma_start(out=xT[:], in_=xT_d.ap().rearrange("(k p) b -> p k b", p=P))

        G1sb = const.tile([P, DK, GD], f32r)
        nc.sync.dma_start(out=G1sb[:], in_=G1_d.ap().rearrange("(k p) m -> p k m", p=P))
        g1sb = const.tile([P, GDK], f32)
        nc.sync.dma_start(out=g1sb[:], in_=g1_d.ap().rearrange("(m p) -> p m", p=P))
        G2sb = const.tile([P, GDK, GD], f32r)
        nc.sync.dma_start(out=G2sb[:], in_=G2_d.ap().rearrange("(k p) m -> p k m", p=P))
        g2sb = const.tile([P, GDK], f32)
        nc.sync.dma_start(out=g2sb[:], in_=g2_d.ap().rearrange("(m p) -> p m", p=P))
        G3dsb = const.tile([P, GDK, 2 * E], f32r)
        nc.sync.dma_start(out=G3dsb[:], in_=G3d_d.ap().rearrange("(k p) m -> p k m", p=P))
        g3dsb = const.tile([2 * E, 1], f32)
        nc.sync.dma_start(out=g3dsb[:], in_=g3d_d.ap().rearrange("(m u) -> m u", u=1))
        S32sb = const.tile([2 * E, OUT], f32r)
        nc.sync.dma_start(out=S32sb[:], in_=S32_d.ap())
        H32sb = const.tile([2 * E, OUT], f32r)
        nc.sync.dma_start(out=H32sb[:], in_=H32_d.ap())
        b3sb = const.tile([OUT, E], f32)
        nc.sync.dma_start(out=b3sb[:], in_=b3T_d.ap())

        exp2T = const.tile([2 * E, b_loc], f32r)   # exp(logit) duplicated x2
        recip2T = const.tile([OUT, b_loc], f32)   # 1/sum_e exp, on 2 partitions
        eo_all = const.tile([2 * E, b_loc], f32r)  # expert outputs (+b3)

        # ---- gating network (scoped scratch) ----------------------------
        with ExitStack() as gctx:
            gpool = gctx.enter_context(tc.tile_pool(name="gpool", bufs=1))
            gpsum = gctx.enter_context(tc.tile_pool(name="gpsum", bufs=2, space="PSUM"))
            h1T = gpool.tile([P, GDK, b_loc], f32r)
            h2T = gpool.tile([P, GDK, b_loc], f32r)
            for c in range(NCH):
                bs = slice(c * CH, (c + 1) * CH)
                for m in range(GDK):
                    ps = gpsum.tile([P, CH], f32, name="gps")
                    for k in range(DK):
                        mm(ps[:], G1sb[:, k, m * P:(m + 1) * P], xT[:, k, bs],
                           k == 0, k == DK - 1)
                    nc.scalar.activation(h1T[:, m, bs], ps[:], AF.Relu,
                                         bias=g1sb[:, m:m + 1])
                for m in range(GDK):
                    ps = gpsum.tile([P, CH], f32, name="gps")
                    for k in range(GDK):
                        mm(ps[:], G2sb[:, k, m * P:(m + 1) * P], h1T[:, k, bs],
                           k == 0, k == GDK - 1)
                    nc.scalar.activation(h2T[:, m, bs], ps[:], AF.Relu,
                                         bias=g2sb[:, m:m + 1])
                eps = gpsum.tile([2 * E, CH], f32, name="eps", bufs=1)
                for k in range(GDK):
                    mm(eps[:], G3dsb[:, k, :], h2T[:, k, bs], k == 0, k == GDK - 1)
                nc.scalar.activation(exp2T[:, bs], eps[:], AF.Exp,
                                     bias=g3dsb[:, 0:1])
                dps = gpsum.tile([OUT, CH], f32, name="dps", bufs=1)
                mm(dps[:], H32sb[:], exp2T[:, bs], True, True)
                nc.vector.reciprocal(recip2T[:, bs], dps[:])

        # ---- experts -----------------------------------------------------
        wpool1 = ctx.enter_context(tc.tile_pool(name="wpool1", bufs=2))
        wpool2 = ctx.enter_context(tc.tile_pool(name="wpool2", bufs=2))
        apool1 = ctx.enter_context(tc.tile_pool(name="apool1", bufs=2))
        apool2 = ctx.enter_context(tc.tile_pool(name="apool2", bufs=2))
        etpool = ctx.enter_context(tc.tile_pool(name="etpool", bufs=3))
        psA = ctx.enter_context(tc.tile_pool(name="psA", bufs=3, space="PSUM"))
        psB = ctx.enter_context(tc.tile_pool(name="psB", bufs=2, space="PSUM"))
        psC = ctx.enter_context(tc.tile_pool(name="psC", bufs=2, space="PSUM"))

        for e in range(E):
            W1sb = wpool1.tile([P, DK, H], f32r, name="W1sb")
            nc.sync.dma_start(out=W1sb[:],
                              in_=W1_d.ap()[e].rearrange("(k p) h -> p k h", p=P))
            b1sb = wpool1.tile([P, HM], f32, name="b1sb")
            nc.sync.dma_start(out=b1sb[:],
                              in_=b1_d.ap()[e].rearrange("(m p) -> p m", p=P))
            W2sb = wpool2.tile([P, HM, H2], f32r, name="W2sb")
            nc.sync.dma_start(out=W2sb[:],
                              in_=W2_d.ap()[e].rearrange("(k p) n -> p k n", p=P))
            b2sb = wpool2.tile([P, H2M], f32, name="b2sb")
            nc.sync.dma_start(out=b2sb[:],
                              in_=b2_d.ap()[e].rearrange("(m p) -> p m", p=P))
            W3sb = wpool2.tile([P, H2M, OUT], f32r, name="W3sb")
            nc.sync.dma_start(out=W3sb[:],
                              in_=W3_d.ap()[e].rearrange("(k p) o -> p k o", p=P))

            for c in range(NCH):
                bs = slice(c * CH, (c + 1) * CH)
                a1T = apool1.tile([P, HM, CH], f32r, name="a1T")
                for m in range(HM):
                    ps = psA.tile([P, CH], f32, name="psA_t")
                    for k in range(DK):
                        mm(ps[:], W1sb[:, k, m * P:(m + 1) * P], xT[:, k, bs],
                           k == 0, k == DK - 1)
                    nc.scalar.activation(a1T[:, m, :], ps[:], AF.Relu,
                                         bias=b1sb[:, m:m + 1])
                a2T = apool2.tile([P, H2M, CH], f32r, name="a2T")
                for m in range(H2M):
                    ps = psB.tile([P, CH], f32, name="psB_t")
                    for k in range(HM):
                        mm(ps[:], W2sb[:, k, m * P:(m + 1) * P], a1T[:, k, :],
                           k == 0, k == HM - 1)
                    nc.vector.tensor_scalar(a2T[:, m, :], ps[:],
                                            b2sb[:, m:m + 1], 0.0,
                                            ALU.add, ALU.max)
                eps = psC.tile([OUT, CH], f32, name="eo_ps")
                for k in range(H2M):
                    mm(eps[:], W3sb[:, k, :], a2T[:, k, :], k == 0, k == H2M - 1)
                eo_tmp = etpool.tile([OUT, CH], f32r, name="eo_tmp")
                nc.scalar.activation(eo_tmp[:], eps[:], AF.Identity,
                                     bias=b3sb[:, e:e + 1])
                nc.sync.dma_start(out=eo_all[2 * e:2 * e + 2, bs], in_=eo_tmp[:])

        # ---- combine -----------------------------------------------------
        cpool = ctx.enter_context(tc.tile_pool(name="cpool", bufs=2))
        for c in range(NCH):
            bs = slice(c * CH, (c + 1) * CH)
            P32 = cpool.tile([2 * E, CH], f32r, name="P32")
            nc.vector.tensor_mul(P32[:], eo_all[:, bs], exp2T[:, bs])
            po = psC.tile([OUT, CH], f32, name="po", bufs=1)
            mm(po[:], S32sb[:], P32[:], True, True)
            out_sb = cpool.tile([OUT, CH], f32, name="out_sb")
            nc.vector.tensor_mul(out_sb[:], po[:], recip2T[:, bs])
            nc.sync.dma_start(out=out_d.ap().rearrange("b o -> o b")[:, bs],
                              in_=out_sb[:])

    nc.compile()
    return nc


def make_host_inputs(inputs, core):
    """Build the per-core in_map from the full problem inputs."""
    x = inputs["x"]
    r0 = core * B_LOC
    xT = np.ascontiguousarray(x[r0:r0 + B_LOC].T)
    G3d = np.ascontiguousarray(np.repeat(inputs["G3"], 2, axis=1))
    g3d = np.ascontiguousarray(np.repeat(inputs["g3"], 2))
    S32 = np.zeros((2 * E, OUT), np.float32)
    for ee in range(E):
        for o in range(OUT):
            S32[2 * ee + o, o] = 1.0
    H32 = np.full((2 * E, OUT), 0.5, np.float32)
    return {
        "xT": xT,
        "W1": np.ascontiguousarray(inputs["W1"]),
        "b1": np.ascontiguousarray(inputs["b1"]),
        "W2": np.ascontiguousarray(inputs["W2"]),
        "b2": np.ascontiguousarray(inputs["b2"]),
        "W3": np.ascontiguousarray(inputs["W3"]),
        "b3T": np.ascontiguousarray(inputs["b3"].T),
        "G1": np.ascontiguousarray(inputs["G1"]),
        "g1": np.ascontiguousarray(inputs["g1"]),
        "G2": np.ascontiguousarray(inputs["G2"]),
        "g2": np.ascontiguousarray(inputs["g2"]),
        "G3d": G3d,
        "g3d": g3d,
        "S32": S32,
        "H32": H32,
    }


_NC_CACHE = {}


def _get_nc(b_loc=B_LOC):
    if b_loc not in _NC_CACHE:
        _NC_CACHE[b_loc] = build_moe(b_loc)
    return _NC_CACHE[b_loc]


def run_hw(inputs, trace=False, b_loc=B_LOC, n_cores=N_CORES, **kw):
    """Run on hardware; returns (out [B,2], BassKernelResults)."""
    from concourse import bass_utils
    from concourse.bass_interp import get_hw_module

    nc = _get_nc(b_loc)
    in_maps = [make_host_inputs(inputs, c) for c in range(n_cores)]
    old_m = nc.m
    nc.m = get_hw_module(nc.m)
    try:
        res = bass_utils.run_bass_kernel_spmd(
            nc, in_maps, core_ids=list(range(n_cores)), trace=trace, **kw)
    finally:
        nc.m = old_m
    out = np.concatenate([r["out"] for r in res.results], axis=0)
    return out, res


def kernel(**inputs):
    out, _ = run_hw(inputs, trace=False)
    return out.astype(np.float32)


# revision 10
# speedup vs baseline: 1.0618x; 1.0618x over previous
"""Trainium2 Bass kernel for nn_MixtureOfExperts_58265526337941.

MoE: gating MLP (D->GD->GD->E, softmax) + E experts (D->H->H/2->2),
combined with gate weights.  B=16384, D=512, E=16, H=1024, GD=256.

Strategy: data-parallel over the batch across 8 cores (B_loc=2048 each),
all params replicated, no collectives.  On-chip layout is
"feature-transposed": activations live as [feature-partition, batch-free]
tiles so the contraction dim is always on partitions and no transposes
are needed between layers.  x is transposed on the host (part of input
sharding) so the kernel DMAs xT directly.

Expert layer l3 produces eoT [2, batch] per expert; these are gathered
(via SBUF->SBUF DMA partition shift) into eo_all [32, batch] with expert
e at partitions 2e:2e+2.  The gating softmax is computed un-normalized
(exp only, duplicated per output col -> exp2T [32, batch]); the combine
is then:  out[o,b] = (sum_p S32[p,o] * exp2T[p,b]*eo_all[p,b]) * recip[b]
with S32 the 0/1 interleave-selection matrix and recip = 1/sum_e exp.
b3 is folded into the eo evacuation (activation bias), so it is gate-
weighted exactly like the reference.

Matmuls run as float32r (full PE rate at N=512); everything else fp32.
"""

import os
import sys
import numpy as np
from contextlib import ExitStack

for _p in ("/opt/trn_rl_repo",):
    if _p not in sys.path and os.path.isdir(_p):
        sys.path.insert(0, _p)

# Problem constants (hardcoded per contract)
B, D, E, H, GD = 16384, 512, 16, 1024, 256
H2 = H // 2
OUT = 2
N_CORES = 8
B_LOC = B // N_CORES  # 2048
P = 128
CH = 512  # batch chunk = matmul moving free dim = one fp32 PSUM bank


def build_moe(b_loc=B_LOC):
    import concourse.bacc as bacc
    import concourse.mybir as mybir
    from concourse import tile

    f32 = mybir.dt.float32
    f32r = mybir.dt.float32r
    AF = mybir.ActivationFunctionType
    ALU = mybir.AluOpType

    NCH = b_loc // CH
    DK = D // P    # 4  k-tiles over D
    HM = H // P    # 8  m-tiles over H
    H2M = H2 // P  # 4  m-tiles over H2
    GDK = GD // P  # 2

    nc = bacc.Bacc("TRN2", target_bir_lowering=False, debug=False)

    # ---- DRAM I/O -------------------------------------------------------
    xT_d = nc.dram_tensor("xT", [D, b_loc], f32r, kind="ExternalInput")
    W1_d = nc.dram_tensor("W1", [E, D, H], f32r, kind="ExternalInput")
    b1_d = nc.dram_tensor("b1", [E, H], f32, kind="ExternalInput")
    W2_d = nc.dram_tensor("W2", [E, H, H2], f32r, kind="ExternalInput")
    b2_d = nc.dram_tensor("b2", [E, H2], f32, kind="ExternalInput")
    W3_d = nc.dram_tensor("W3", [E, H2, OUT], f32r, kind="ExternalInput")
    b3T_d = nc.dram_tensor("b3T", [OUT, E], f32, kind="ExternalInput")
    G1_d = nc.dram_tensor("G1", [D, GD], f32r, kind="ExternalInput")
    g1_d = nc.dram_tensor("g1", [GD], f32, kind="ExternalInput")
    G2_d = nc.dram_tensor("G2", [GD, GD], f32r, kind="ExternalInput")
    g2_d = nc.dram_tensor("g2", [GD], f32, kind="ExternalInput")
    G3d_d = nc.dram_tensor("G3d", [GD, 2 * E], f32r, kind="ExternalInput")
    g3d_d = nc.dram_tensor("g3d", [2 * E], f32, kind="ExternalInput")
    S32_d = nc.dram_tensor("S32", [2 * E, OUT], f32r, kind="ExternalInput")
    H32_d = nc.dram_tensor("H32", [2 * E, OUT], f32r, kind="ExternalInput")
    # [OUT, b_loc] so the store DMA is contiguous; host transposes back.
    out_d = nc.dram_tensor("out", [OUT, b_loc], f32, kind="ExternalOutput")

    def mm(ps, lhsT, rhs, start, stop):
        nc.tensor.matmul(ps, lhsT.bitcast(f32r), rhs.bitcast(f32r),
                         start=start, stop=stop)

    with tile.TileContext(nc) as tc, ExitStack() as ctx:
        const = ctx.enter_context(tc.tile_pool(name="const", bufs=1))

        # Persistent tiles.  DMA order matters: the first gating matmul
        # needs only G1sb + xT chunk 0, so issue those first and stream
        # the remaining xT chunks behind them.
        G1sb = const.tile([P, DK, GD], f32r)
        nc.sync.dma_start(out=G1sb[:], in_=G1_d.ap().rearrange("(k p) m -> p k m", p=P))
        g1sb = const.tile([P, GDK], f32)
        nc.sync.dma_start(out=g1sb[:], in_=g1_d.ap().rearrange("(m p) -> p m", p=P))
        xT = const.tile([P, DK, b_loc], f32r)
        xT_src = xT_d.ap().rearrange("(k p) b -> p k b", p=P)
        nc.sync.dma_start(out=xT[:, :, 0:CH], in_=xT_src[:, :, 0:CH])
        G2sb = const.tile([P, GDK, GD], f32r)
        nc.sync.dma_start(out=G2sb[:], in_=G2_d.ap().rearrange("(k p) m -> p k m", p=P))
        g2sb = const.tile([P, GDK], f32)
        nc.sync.dma_start(out=g2sb[:], in_=g2_d.ap().rearrange("(m p) -> p m", p=P))
        G3dsb = const.tile([P, GDK, 2 * E], f32r)
        nc.sync.dma_start(out=G3dsb[:], in_=G3d_d.ap().rearrange("(k p) m -> p k m", p=P))
        g3dsb = const.tile([2 * E, 1], f32)
        nc.sync.dma_start(out=g3dsb[:], in_=g3d_d.ap().rearrange("(m u) -> m u", u=1))
        S32sb = const.tile([2 * E, OUT], f32r)
        nc.sync.dma_start(out=S32sb[:], in_=S32_d.ap())
        H32sb = const.tile([2 * E, OUT], f32r)
        nc.sync.dma_start(out=H32sb[:], in_=H32_d.ap())
        b3sb = const.tile([OUT, E], f32)
        nc.sync.dma_start(out=b3sb[:], in_=b3T_d.ap())
        for c in range(1, NCH):
            cs = slice(c * CH, (c + 1) * CH)
            nc.sync.dma_start(out=xT[:, :, cs], in_=xT_src[:, :, cs])

        exp2T = const.tile([2 * E, b_loc], f32r)   # exp(logit) duplicated x2
        recip2T = const.tile([OUT, b_loc], f32)   # 1/sum_e exp, on 2 partitions
        eo_all = const.tile([2 * E, b_loc], f32r)  # expert outputs (+b3)

        # psD holds the softmax-denominator + combine-output PSUM tiles in
        # a dedicated bank so the slow reciprocals at gating end don't
        # block the expert-phase PSUM pools via bank-overlap WAR deps.
        psD = ctx.enter_context(tc.tile_pool(name="psD", bufs=1, space="PSUM"))

        # ---- gating network (scoped scratch) ----------------------------
        with ExitStack() as gctx:
            gpool = gctx.enter_context(tc.tile_pool(name="gpool", bufs=1))
            gpsum = gctx.enter_context(tc.tile_pool(name="gpsum", bufs=2, space="PSUM"))
            h1T = gpool.tile([P, GDK, b_loc], f32r)
            h2T = gpool.tile([P, GDK, b_loc], f32r)
            for c in range(NCH):
                bs = slice(c * CH, (c + 1) * CH)
                for m in range(GDK):
                    ps = gpsum.tile([P, CH], f32, name="gps")
                    for k in range(DK):
                        mm(ps[:], G1sb[:, k, m * P:(m + 1) * P], xT[:, k, bs],
                           k == 0, k == DK - 1)
                    nc.scalar.activation(h1T[:, m, bs], ps[:], AF.Relu,
                                         bias=g1sb[:, m:m + 1])
                for m in range(GDK):
                    ps = gpsum.tile([P, CH], f32, name="gps")
                    for k in range(GDK):
                        mm(ps[:], G2sb[:, k, m * P:(m + 1) * P], h1T[:, k, bs],
                           k == 0, k == GDK - 1)
                    nc.scalar.activation(h2T[:, m, bs], ps[:], AF.Relu,
                                         bias=g2sb[:, m:m + 1])
                eps = gpsum.tile([2 * E, CH], f32, name="eps", bufs=1)
                for k in range(GDK):
                    mm(eps[:], G3dsb[:, k, :], h2T[:, k, bs], k == 0, k == GDK - 1)
                nc.scalar.activation(exp2T[:, bs], eps[:], AF.Exp,
                                     bias=g3dsb[:, 0:1])
                dps = psD.tile([OUT, CH], f32, name="dps")
                mm(dps[:], H32sb[:], exp2T[:, bs], True, True)
                nc.vector.reciprocal(recip2T[:, bs], dps[:])

        # ---- experts -----------------------------------------------------
        wpool1 = ctx.enter_context(tc.tile_pool(name="wpool1", bufs=2))
        wpool2 = ctx.enter_context(tc.tile_pool(name="wpool2", bufs=2))
        apool1 = ctx.enter_context(tc.tile_pool(name="apool1", bufs=2))
        apool2 = ctx.enter_context(tc.tile_pool(name="apool2", bufs=2))
        etpool = ctx.enter_context(tc.tile_pool(name="etpool", bufs=3))
        psA = ctx.enter_context(tc.tile_pool(name="psA", bufs=3, space="PSUM"))
        psB = ctx.enter_context(tc.tile_pool(name="psB", bufs=2, space="PSUM"))
        psC = ctx.enter_context(tc.tile_pool(name="psC", bufs=2, space="PSUM"))

        for e in range(E):
            W1sb = wpool1.tile([P, DK, H], f32r, name="W1sb")
            nc.sync.dma_start(out=W1sb[:],
                              in_=W1_d.ap()[e].rearrange("(k p) h -> p k h", p=P))
            b1sb = wpool1.tile([P, HM], f32, name="b1sb")
            nc.sync.dma_start(out=b1sb[:],
                              in_=b1_d.ap()[e].rearrange("(m p) -> p m", p=P))
            W2sb = wpool2.tile([P, HM, H2], f32r, name="W2sb")
            nc.sync.dma_start(out=W2sb[:],
                              in_=W2_d.ap()[e].rearrange("(k p) n -> p k n", p=P))
            b2sb = wpool2.tile([P, H2M], f32, name="b2sb")
            nc.sync.dma_start(out=b2sb[:],
                              in_=b2_d.ap()[e].rearrange("(m p) -> p m", p=P))
            W3sb = wpool2.tile([P, H2M, OUT], f32r, name="W3sb")
            nc.sync.dma_start(out=W3sb[:],
                              in_=W3_d.ap()[e].rearrange("(k p) o -> p k o", p=P))

            for c in range(NCH):
                bs = slice(c * CH, (c + 1) * CH)
                a1T = apool1.tile([P, HM, CH], f32r, name="a1T")
                for m in range(HM):
                    ps = psA.tile([P, CH], f32, name="psA_t")
                    for k in range(DK):
                        mm(ps[:], W1sb[:, k, m * P:(m + 1) * P], xT[:, k, bs],
                           k == 0, k == DK - 1)
                    nc.scalar.activation(a1T[:, m, :], ps[:], AF.Relu,
                                         bias=b1sb[:, m:m + 1])
                a2T = apool2.tile([P, H2M, CH], f32r, name="a2T")
                for m in range(H2M):
                    ps = psB.tile([P, CH], f32, name="psB_t")
                    for k in range(HM):
                        mm(ps[:], W2sb[:, k, m * P:(m + 1) * P], a1T[:, k, :],
                           k == 0, k == HM - 1)
                    nc.vector.tensor_scalar(a2T[:, m, :], ps[:],
                                            b2sb[:, m:m + 1], 0.0,
                                            ALU.add, ALU.max)
                eps = psC.tile([OUT, CH], f32, name="eo_ps")
                for k in range(H2M):
                    mm(eps[:], W3sb[:, k, :], a2T[:, k, :], k == 0, k == H2M - 1)
                eo_tmp = etpool.tile([OUT, CH], f32r, name="eo_tmp")
                nc.scalar.activation(eo_tmp[:], eps[:], AF.Identity,
                                     bias=b3sb[:, e:e + 1])
                nc.sync.dma_start(out=eo_all[2 * e:2 * e + 2, bs], in_=eo_tmp[:])

        # ---- combine -----------------------------------------------------
        cpool = ctx.enter_context(tc.tile_pool(name="cpool", bufs=2))
        for c in range(NCH):
            bs = slice(c * CH, (c + 1) * CH)
            P32 = cpool.tile([2 * E, CH], f32r, name="P32")
            nc.vector.tensor_mul(P32[:], eo_all[:, bs], exp2T[:, bs])
            po = psD.tile([OUT, CH], f32, name="dps")
            mm(po[:], S32sb[:], P32[:], True, True)
            out_sb = cpool.tile([OUT, CH], f32, name="out_sb")
            nc.vector.tensor_mul(out_sb[:], po[:], recip2T[:, bs])
            nc.sync.dma_start(out=out_d.ap()[:, bs], in_=out_sb[:])

    nc.compile()
    return nc


def make_host_inputs(inputs, core):
    """Build the per-core in_map from the full problem inputs."""
    x = inputs["x"]
    r0 = core * B_LOC
    xT = np.ascontiguousarray(x[r0:r0 + B_LOC].T)
    G3d = np.ascontiguousarray(np.repeat(inputs["G3"], 2, axis=1))
    g3d = np.ascontiguousarray(np.repeat(inputs["g3"], 2))
    S32 = np.zeros((2 * E, OUT), np.float32)
    for ee in range(E):
        for o in range(OUT):
            S32[2 * ee + o, o] = 1.0
    H32 = np.full((2 * E, OUT), 0.5, np.float32)
    return {
        "xT": xT,
        "W1": np.ascontiguousarray(inputs["W1"]),
        "b1": np.ascontiguousarray(inputs["b1"]),
        "W2": np.ascontiguousarray(inputs["W2"]),
        "b2": np.ascontiguousarray(inputs["b2"]),
        "W3": np.ascontiguousarray(inputs["W3"]),
        "b3T": np.ascontiguousarray(inputs["b3"].T),
        "G1": np.ascontiguousarray(inputs["G1"]),
        "g1": np.ascontiguousarray(inputs["g1"]),
        "G2": np.ascontiguousarray(inputs["G2"]),
        "g2": np.ascontiguousarray(inputs["g2"]),
        "G3d": G3d,
        "g3d": g3d,
        "S32": S32,
        "H32": H32,
    }


_NC_CACHE = {}


def _get_nc(b_loc=B_LOC):
    if b_loc not in _NC_CACHE:
        _NC_CACHE[b_loc] = build_moe(b_loc)
    return _NC_CACHE[b_loc]


def run_hw(inputs, trace=False, b_loc=B_LOC, n_cores=N_CORES, **kw):
    """Run on hardware; returns (out [B,2], BassKernelResults)."""
    from concourse import bass_utils
    from concourse.bass_interp import get_hw_module

    nc = _get_nc(b_loc)
    in_maps = [make_host_inputs(inputs, c) for c in range(n_cores)]
    old_m = nc.m
    nc.m = get_hw_module(nc.m)
    try:
        res = bass_utils.run_bass_kernel_spmd(
            nc, in_maps, core_ids=list(range(n_cores)), trace=trace, **kw)
    finally:
        nc.m = old_m
    out = np.concatenate([r["out"].T for r in res.results], axis=0)
    return out, res


def kernel(**inputs):
    out, _ = run_hw(inputs, trace=False)
    return out.astype(np.float32)
